# revision 12
# baseline (speedup 1.0000x reference)
"""MPNN (2x NNConv + BN + global mean pool + MLP) on 8 Trainium2 cores.

Strategy (node-sharded message passing):
  * Never materialize We=[E,in_c,out_c].  msg[e] = (z[e] (x) xs[e]) @ W2r
    where z=relu(ea@W1+b1), xs=x[src], W2r = reshape of W2.  Since the
    segment-sum over dst commutes with the (shared) @W2r, we scatter the
    per-edge outer products u[e]=(z (x) xs_scaled) into per-node U first,
    then do ONE matmul per node tile:  agg = U @ W2r  (3x fewer MACs).
  * Nodes are bin-packed into 128-node windows balanced by edge count;
    each core owns 10 windows.  Edges are grouped per window (padded to
    T tiles of 128).  Scatter = one-hot matmul on the PE accumulating
    U^T chunks directly in PSUM (no transposes needed anywhere in the
    scatter->agg path).
  * BN stats via ones-matmul, tiny AllReduce; normalize via transpose
    sandwich.  h1 slices AllGathered for layer-2 gathers.  Pool via
    one-hot matmul over graphs + AllReduce; final MLP replicated.
"""

import sys

import numpy as np

try:
    import concourse.bass as bass  # noqa
except Exception:  # pragma: no cover
    sys.path.insert(0, "/opt/trn_rl_repo")

import concourse.bacc as bacc
import concourse.bass as bass
import concourse.mybir as mybir
import concourse.tile as tile
from concourse.bass import IndirectOffsetOnAxis
from concourse.bass_utils import run_bass_kernel_spmd
from concourse.masks import make_identity

P = 128
NCORES = 8
N = 10000
E = 30000
NG = 256
IN_C = 16
EDGE_C = 8
KH = 32  # edge-MLP hidden width
H1 = 64  # conv1 out channels
H2 = 128  # conv2 out channels
WPC = 10  # windows per core
NPADC = WPC * P  # padded nodes per core (1280)
EPS = 1e-5
f32 = mybir.dt.float32
i32 = mybir.dt.int32

U1 = KH * IN_C  # 512
U1E = 640  # padded to 5 chunks of 128 (u | xs | zeros)
U2 = KH * H1  # 2048
U2E = 2176  # padded to 17 chunks of 128 (u | xs | zeros)


# --------------------------------------------------------------------------
# host-side preprocessing: index/layout work only (no float math on inputs
# beyond transposes/reshapes of weights and 1/count tables)
# --------------------------------------------------------------------------
def _preprocess(x, edge_index, edge_attr, batch):
    import heapq

    src = np.asarray(edge_index[0], dtype=np.int64)
    dst = np.asarray(edge_index[1], dtype=np.int64)
    deg = np.bincount(dst, minlength=N).astype(np.int64)

    # ---- bin-pack nodes into NCORES*WPC windows of exactly <=128 nodes,
    # balancing per-window edge counts (LPT greedy) ----
    NW = NCORES * WPC
    order = np.argsort(-deg, kind="stable")
    wsum = np.zeros(NW, dtype=np.int64)
    wcnt = np.zeros(NW, dtype=np.int64)
    win_of = np.empty(N, dtype=np.int64)
    slot_of = np.empty(N, dtype=np.int64)
    heap = [(0, w) for w in range(NW)]
    heapq.heapify(heap)
    for n in order:
        while True:
            _, w = heapq.heappop(heap)
            if wcnt[w] < P:
                break
        win_of[n] = w
        slot_of[n] = wcnt[w]
        wcnt[w] += 1
        wsum[w] += deg[n]
        if wcnt[w] < P:
            heapq.heappush(heap, (int(wsum[w]), w))

    T = max(1, int(-(-int(wsum.max()) // P)))  # tiles (of 128 edges) per window
    ES = WPC * T * P  # edge slots per core

    core_of = win_of // WPC
    pos = core_of * NPADC + (win_of % WPC) * P + slot_of  # padded global slot

    # ---- per-edge placement ----
    ew = win_of[dst]
    eorder = np.argsort(ew, kind="stable")
    inv_cnt = 1.0 / np.maximum(deg, 1).astype(np.float32)

    ea_s = np.zeros((NCORES, ES, EDGE_C), dtype=np.float32)
    ones_s = np.zeros((NCORES, ES), dtype=np.float32)
    srcx_s = np.zeros((NCORES, ES), dtype=np.int32)
    srch_s = np.zeros((NCORES, ES), dtype=np.int32)
    dstrel_s = np.full((NCORES, ES), -1.0, dtype=np.float32)
    icnt_s = np.zeros((NCORES, ES), dtype=np.float32)

    ew_sorted = ew[eorder]
    # slot ranges per window
    starts = np.searchsorted(ew_sorted, np.arange(NW))
    ends = np.searchsorted(ew_sorted, np.arange(NW) + 1)
    ea_np = np.asarray(edge_attr, dtype=np.float32)
    for w in range(NW):
        es = eorder[starts[w] : ends[w]]
        c = w // WPC
        base = (w % WPC) * T * P
        k = len(es)
        assert k <= T * P
        sl = slice(base, base + k)
        ea_s[c, sl] = ea_np[es]
        ones_s[c, sl] = 1.0
        srcx_s[c, sl] = src[es]
        srch_s[c, sl] = pos[src[es]]
        dstrel_s[c, sl] = slot_of[dst[es]]
        icnt_s[c, sl] = inv_cnt[dst[es]]

    # eaT_aug: [9, ES] per core (edge_attr^T with a ones row for the bias)
    eaT = np.concatenate(
        [np.transpose(ea_s, (0, 2, 1)), ones_s[:, None, :]], axis=1
    ).astype(np.float32)

    # ---- per-node per-core tables ----
    batch = np.asarray(batch, dtype=np.int64)
    gcnt = np.bincount(batch, minlength=NG).astype(np.int64)
    igc_node = (1.0 / np.maximum(gcnt, 1).astype(np.float32))[batch]

    xT_s = np.zeros((NCORES, IN_C, NPADC), dtype=np.float32)
    batch_s = np.full((NCORES, NPADC), -1.0, dtype=np.float32)
    igc_s = np.zeros((NCORES, NPADC), dtype=np.float32)
    vmask_s = np.zeros((NCORES, NPADC), dtype=np.float32)
    lpos = pos % NPADC
    x_np = np.asarray(x, dtype=np.float32)
    for c in range(NCORES):
        m = core_of == c
        xT_s[c][:, lpos[m]] = x_np[m].T
        batch_s[c][lpos[m]] = batch[m].astype(np.float32)
        igc_s[c][lpos[m]] = igc_node[m]
        vmask_s[c][lpos[m]] = 1.0

    return dict(
        T=T, ES=ES, eaT=eaT, srcx=srcx_s, srch=srch_s, dstrel=dstrel_s,
        icnt=icnt_s, xT=xT_s, batchrel=batch_s, igc=igc_s, vmask=vmask_s,
    )


def _weights(p):
    w = {}
    w["W1a1"] = np.concatenate([p["nn1_W1"], p["nn1_b1"][None, :]], 0).astype(np.float32)
    w["W1a2"] = np.concatenate([p["nn2_W1"], p["nn2_b1"][None, :]], 0).astype(np.float32)
    wp1 = np.zeros((U1E, H1), dtype=np.float32)
    wp1[:U1] = p["nn1_W2"].reshape(KH, IN_C, H1).reshape(U1, H1)
    wp1[U1 : U1 + IN_C] = p["nn1_b2"].reshape(IN_C, H1)
    w["Wp1"] = wp1
    wp2 = np.zeros((U2E, H2), dtype=np.float32)
    wp2[:U2] = p["nn2_W2"].reshape(KH, H1, H2).reshape(U2, H2)
    wp2[U2 : U2 + H1] = p["nn2_b2"].reshape(H1, H2)
    w["Wp2"] = wp2
    w["root1"] = np.asarray(p["root1"], np.float32)
    w["root2"] = np.asarray(p["root2"], np.float32)
    w["bias1r"] = np.asarray(p["bias1"], np.float32)[None, :]
    w["bias2r"] = np.asarray(p["bias2"], np.float32)[None, :]
    w["bng1"] = np.asarray(p["bn1_g"], np.float32)[:, None]
    w["bnb1"] = np.asarray(p["bn1_b"], np.float32)[:, None]
    w["bng2"] = np.asarray(p["bn2_g"], np.float32)[:, None]
    w["bnb2"] = np.asarray(p["bn2_b"], np.float32)[:, None]
    w["l1W"] = np.asarray(p["lin1_W"], np.float32)
    w["l1b"] = np.asarray(p["lin1_b"], np.float32)[:, None]
    w["l2W"] = np.asarray(p["lin2_W"], np.float32)
    w["l2b"] = np.asarray(p["lin2_b"], np.float32)[None, :]
    w["iota128"] = np.broadcast_to(np.arange(P, dtype=np.float32), (P, P)).copy()
    w["iota256"] = np.broadcast_to(np.arange(NG, dtype=np.float32), (P, NG)).copy()
    w["onesP"] = np.ones((P, 1), dtype=np.float32)
    w["onesr"] = np.ones((1, P), dtype=np.float32)
    return w


# --------------------------------------------------------------------------
# device program (identical for all cores; per-core data comes via inputs)
# --------------------------------------------------------------------------
def build_program(T, ES):
    AL = mybir.AluOpType
    AF = mybir.ActivationFunctionType
    nc = bacc.Bacc("TRN2", target_bir_lowering=False, debug=False, num_devices=NCORES)

    def din(name, shape, dtype=f32):
        return nc.dram_tensor(name, shape, dtype, kind="ExternalInput").ap()

    eaT_d = din("eaT", [EDGE_C + 1, ES])
    srcx_d = din("srcx", [ES, 1], i32)
    srch_d = din("srch", [ES, 1], i32)
    dstrel_d = din("dstrel", [ES, 1])
    icnt_d = din("icnt", [ES, 1])
    x_d = din("x", [N, IN_C])
    xT_d = din("xT", [IN_C, NPADC])
    batch_d = din("batchrel", [NPADC, 1])
    igc_d = din("igc", [NPADC, 1])
    vmask_d = din("vmask", [NPADC, 1])
    W1a1_d = din("W1a1", [EDGE_C + 1, KH])
    W1a2_d = din("W1a2", [EDGE_C + 1, KH])
    Wp1_d = din("Wp1", [U1E, H1])
    Wp2_d = din("Wp2", [U2E, H2])
    root1_d = din("root1", [IN_C, H1])
    root2_d = din("root2", [H1, H2])
    bias1r_d = din("bias1r", [1, H1])
    bias2r_d = din("bias2r", [1, H2])
    bng1_d = din("bng1", [H1, 1])
    bnb1_d = din("bnb1", [H1, 1])
    bng2_d = din("bng2", [H2, 1])
    bnb2_d = din("bnb2", [H2, 1])
    l1W_d = din("l1W", [H2, H1])
    l1b_d = din("l1b", [H1, 1])
    l2W_d = din("l2W", [H1, 1])
    l2b_d = din("l2b", [1, 1])
    iota128_d = din("iota128", [P, P])
    iota256_d = din("iota256", [P, NG])
    onesP_d = din("onesP", [P, 1])
    onesr_d = din("onesr", [1, P])
    out_d = nc.dram_tensor("out", [1, NG], f32, kind="ExternalOutput").ap()

    NC1 = U1E // P  # 5 chunks
    NC2 = U2E // P  # 17 chunks

    from contextlib import ExitStack

    with tile.TileContext(nc) as tc, ExitStack() as pools:
        cst = pools.enter_context(tc.tile_pool(name="cst", bufs=1))
        sb = pools.enter_context(tc.tile_pool(name="sb", bufs=3))
        stash = pools.enter_context(tc.tile_pool(name="stash", bufs=WPC))
        pp_u = pools.enter_context(tc.tile_pool(name="pp_u", bufs=1, space="PSUM"))
        pp_pre = pools.enter_context(tc.tile_pool(name="pp_pre", bufs=1, space="PSUM"))
        pp_z = pools.enter_context(tc.tile_pool(name="pp_z", bufs=1, space="PSUM"))
        pp_s = pools.enter_context(tc.tile_pool(name="pp_s", bufs=1, space="PSUM"))
        dram = pools.enter_context(tc.tile_pool(name="dram", bufs=1, space="DRAM"))

        # ---- resident constants ----
        def load(shape, ap, name):
            t = cst.tile(shape, f32, tag=name)
            nc.sync.dma_start(out=t[:], in_=ap)
            return t

        ident = cst.tile([P, P], f32, tag="ident")
        make_identity(nc, ident[:])
        W1a1 = load([EDGE_C + 1, KH], W1a1_d[:], "W1a1")
        W1a2 = load([EDGE_C + 1, KH], W1a2_d[:], "W1a2")
        Wp1 = cst.tile([P, NC1, H1], f32, tag="Wp1")
        nc.sync.dma_start(out=Wp1[:], in_=Wp1_d.rearrange("(c p) o -> p c o", p=P))
        Wp2 = cst.tile([P, NC2, H2], f32, tag="Wp2")
        nc.sync.dma_start(out=Wp2[:], in_=Wp2_d.rearrange("(c p) o -> p c o", p=P))
        xT = load([IN_C, NPADC], xT_d[:], "xT")
        root1 = load([IN_C, H1], root1_d[:], "root1")
        root2 = load([H1, H2], root2_d[:], "root2")
        bias1r = load([1, H1], bias1r_d[:], "bias1r")
        bias2r = load([1, H2], bias2r_d[:], "bias2r")
        bng1 = load([H1, 1], bng1_d[:], "bng1")
        bnb1 = load([H1, 1], bnb1_d[:], "bnb1")
        bng2 = load([H2, 1], bng2_d[:], "bng2")
        bnb2 = load([H2, 1], bnb2_d[:], "bnb2")
        l1W = load([H2, H1], l1W_d[:], "l1W")
        l1b = load([H1, 1], l1b_d[:], "l1b")
        l2W = load([H1, 1], l2W_d[:], "l2W")
        l2b = load([1, 1], l2b_d[:], "l2b")
        iota128 = load([P, P], iota128_d[:], "iota128")
        iota256 = load([P, NG], iota256_d[:], "iota256")
        onesP = load([P, 1], onesP_d[:], "onesP")
        onesr = load([1, P], onesr_d[:], "onesr")
        # per-node tables as [slot(partition), window(free)]
        vmask = load([P, WPC], vmask_d.rearrange("(w s) o -> s (w o)", s=P), "vmask")
        igc = load([P, WPC], igc_d.rearrange("(w s) o -> s (w o)", s=P), "igc")
        batchrel = load([P, WPC], batch_d.rearrange("(w s) o -> s (w o)", s=P), "batchrel")

        stats_sb1 = cst.tile([H1, 2], f32, tag="st1")
        stats_sb2 = cst.tile([H2, 2], f32, tag="st2")
        gT_sb = cst.tile([P, NG], f32, tag="gT")
        nc.vector.memset(stats_sb1[:], 0.0)
        nc.vector.memset(stats_sb2[:], 0.0)
        nc.vector.memset(gT_sb[:], 0.0)

        h1_slice = dram.tile([NPADC, H1], f32, tag="h1s")
        h1_full = dram.tile([NCORES * NPADC, H1], f32, tag="h1f")
        st1_loc = dram.tile([H1, 2], f32, tag="st1l")
        st1_g = dram.tile([H1, 2], f32, tag="st1g")
        st2_loc = dram.tile([H2, 2], f32, tag="st2l")
        st2_g = dram.tile([H2, 2], f32, tag="st2g")
        gT_loc = dram.tile([P, NG], f32, tag="gTl")
        gT_g = dram.tile([P, NG], f32, tag="gTg")

        groups = [list(range(NCORES))]

        # ================= generic conv layer =================
        def conv_layer(layer):
            if layer == 1:
                inc, outc, UE, NCH, W1a, Wp = IN_C, H1, U1E, NC1, W1a1, Wp1
                uw = U1
            else:
                inc, outc, UE, NCH, W1a, Wp = H1, H2, U2E, NC2, W1a2, Wp2
                uw = U2
            pre_list = []
            for w in range(WPC):
                UT = pp_u.tile([P, NCH, P], f32, tag="ut")
                u_tiles = []
                oh_tiles = []
                for t3 in range(T):
                    t = w * T + t3
                    s0 = t * P
                    ea_t = sb.tile([EDGE_C + 1, P], f32, tag="ea")
                    nc.sync.dma_start(out=ea_t[:], in_=eaT_d[:, s0 : s0 + P])
                    idx = sb.tile([P, 1], i32, tag="idx")
                    nc.sync.dma_start(
                        out=idx[:],
                        in_=(srcx_d if layer == 1 else srch_d)[s0 : s0 + P, :],
                    )
                    drel = sb.tile([P, 1], f32, tag="drel")
                    nc.sync.dma_start(out=drel[:], in_=dstrel_d[s0 : s0 + P, :])
                    ic_t = sb.tile([P, 1], f32, tag="ic")
                    nc.sync.dma_start(out=ic_t[:], in_=icnt_d[s0 : s0 + P, :])

                    # z = relu(ea @ W1 + b1)
                    zp = pp_z.tile([P, KH], f32, tag="z")
                    nc.tensor.matmul(out=zp[:], lhsT=ea_t[:], rhs=W1a[:], start=True, stop=True)
                    z = sb.tile([P, KH], f32, tag="z_sb")
                    nc.vector.tensor_scalar_max(out=z[:], in0=zp[:], scalar1=0.0)

                    # gather xs and scale by 1/cnt(dst)
                    xs = sb.tile([P, inc], f32, tag="xs")
                    nc.gpsimd.indirect_dma_start(
                        out=xs[:],
                        out_offset=None,
                        in_=(x_d if layer == 1 else h1_full[:]),
                        in_offset=IndirectOffsetOnAxis(ap=idx[:, :1], axis=0),
                    )
                    xss = sb.tile([P, inc], f32, tag="xss")
                    nc.vector.tensor_scalar_mul(out=xss[:], in0=xs[:], scalar1=ic_t[:, :1])

                    # u = z (x) xss  -> [P, uw], plus xss block, plus zero pad
                    u = sb.tile([P, UE], f32, tag="u")
                    nc.vector.tensor_tensor(
                        out=u[:, :uw].rearrange("p (k i) -> p k i", k=KH),
                        in0=z[:].unsqueeze(2).to_broadcast([P, KH, inc]),
                        in1=xss[:].unsqueeze(1).to_broadcast([P, KH, inc]),
                        op=AL.mult,
                    )
                    nc.scalar.copy(out=u[:, uw : uw + inc], in_=xss[:])
                    nc.gpsimd.memset(u[:, uw + inc :], 0.0)

                    # one-hot over window slots
                    oh = sb.tile([P, P], f32, tag="oh")
                    nc.vector.tensor_scalar(
                        out=oh[:], in0=iota128[:], scalar1=drel[:, :1],
                        scalar2=None, op0=AL.is_equal,
                    )
                    u_tiles.append(u)
                    oh_tiles.append(oh)
                # scatter: chunk-outer so each PSUM accumulation group is
                # open-close before the next one starts
                for c in range(NCH):
                    for t3 in range(T):
                        nc.tensor.matmul(
                            out=UT[:, c, :],
                            lhsT=u_tiles[t3][:, c * P : (c + 1) * P],
                            rhs=oh_tiles[t3][:],
                            start=(t3 == 0),
                            stop=(t3 == T - 1),
                        )

                # ---- node phase for window w ----
                UTs = sb.tile([P, NCH, P], f32, tag="uts")
                nc.vector.tensor_copy(out=UTs[:], in_=UT[:])
                pre = pp_pre.tile([P, outc], f32, tag="pre")
                for c in range(NCH):
                    nc.tensor.matmul(
                        out=pre[:], lhsT=UTs[:, c, :], rhs=Wp[:, c, :],
                        start=(c == 0), stop=False,
                    )
                if layer == 1:
                    nc.tensor.matmul(
                        out=pre[:], lhsT=xT[:, w * P : (w + 1) * P], rhs=root1[:],
                        start=False, stop=False,
                    )
                else:
                    nc.tensor.matmul(
                        out=pre[:], lhsT=h1T_list[w][:], rhs=root2[:],
                        start=False, stop=False,
                    )
                nc.tensor.matmul(
                    out=pre[:], lhsT=onesr[:], rhs=(bias1r if layer == 1 else bias2r)[:],
                    start=False, stop=True,
                )
                # relu then mask out pad nodes
                pre_sb = stash.tile([P, outc], f32, tag=f"pre{layer}")
                nc.vector.tensor_scalar(
                    out=pre_sb[:], in0=pre[:], scalar1=0.0,
                    scalar2=vmask[:, w : w + 1], op0=AL.max, op1=AL.mult,
                )
                pre_list.append(pre_sb)
                # bn stats partial sums (into SBUF accumulators)
                sq = sb.tile([P, outc], f32, tag="sq")
                nc.scalar.activation(out=sq[:], in_=pre_sb[:], func=AF.Square)
                stp = pp_s.tile([outc, 2], f32, tag="st")
                nc.tensor.matmul(out=stp[:, 0:1], lhsT=pre_sb[:], rhs=onesP[:], start=True, stop=True)
                nc.tensor.matmul(out=stp[:, 1:2], lhsT=sq[:], rhs=onesP[:], start=True, stop=True)
                st_sb = stats_sb1 if layer == 1 else stats_sb2
                nc.vector.tensor_add(out=st_sb[:], in0=st_sb[:], in1=stp[:])
            return pre_list

        # ================= bn scale/shift from stats =================
        def bn_coeffs(st_sb, loc, glob, outc, bng, bnb):
            nc.sync.dma_start(out=loc[:], in_=st_sb[:])
            nc.gpsimd.collective_compute(
                "AllReduce", mybir.AluOpType.add, replica_groups=groups,
                ins=[loc.opt()], outs=[glob.opt()],
            )
            stg = sb.tile([outc, 2], f32, tag="stg")
            nc.sync.dma_start(out=stg[:], in_=glob[:])
            mu = sb.tile([outc, 1], f32, tag="mu")
            nc.vector.tensor_scalar_mul(out=mu[:], in0=stg[:, 0:1], scalar1=1.0 / N)
            va = sb.tile([outc, 1], f32, tag="va")
            # va = E[x^2] - mu^2
            nc.vector.tensor_scalar_mul(out=va[:], in0=stg[:, 1:2], scalar1=1.0 / N)
            musq = sb.tile([outc, 1], f32, tag="musq")
            nc.vector.tensor_mul(out=musq[:], in0=mu[:], in1=mu[:])
            nc.vector.tensor_sub(out=va[:], in0=va[:], in1=musq[:])
            nc.vector.tensor_scalar_add(out=va[:], in0=va[:], scalar1=EPS)
            sd = sb.tile([outc, 1], f32, tag="sd")
            nc.scalar.activation(out=sd[:], in_=va[:], func=AF.Sqrt)
            rs = sb.tile([outc, 1], f32, tag="rs")
            nc.vector.reciprocal(out=rs[:], in_=sd[:])
            scale = sb.tile([outc, 1], f32, tag="scale")
            nc.vector.tensor_mul(out=scale[:], in0=rs[:], in1=bng[:])
            shift = sb.tile([outc, 1], f32, tag="shift")
            nc.vector.tensor_mul(out=shift[:], in0=mu[:], in1=scale[:])
            nc.vector.tensor_sub(out=shift[:], in0=bnb[:], in1=shift[:])
            return scale, shift

        # ======================= layer 1 =======================
        pre1 = conv_layer(1)
        sc1, sh1 = bn_coeffs(stats_sb1, st1_loc, st1_g, H1, bng1, bnb1)
        h1T_list = []
        for w in range(WPC):
            tp = pp_u.tile([H1, P], f32, tag="ut")
            nc.tensor.transpose(out=tp[:], in_=pre1[w][:], identity=ident[:])
            h1T = stash.tile([H1, P], f32, tag="h1T")
            nc.vector.tensor_scalar(
                out=h1T[:], in0=tp[:], scalar1=sc1[:, :1], scalar2=sh1[:, :1],
                op0=AL.mult, op1=AL.add,
            )
            h1T_list.append(h1T)
            bk = pp_pre.tile([P, H1], f32, tag="pre")
            nc.tensor.transpose(out=bk[:], in_=h1T[:], identity=ident[:H1, :H1])
            h1n = sb.tile([P, H1], f32, tag="h1n")
            nc.vector.tensor_copy(out=h1n[:], in_=bk[:])
            nc.sync.dma_start(out=h1_slice[w * P : (w + 1) * P, :], in_=h1n[:])
        nc.gpsimd.collective_compute(
            "AllGather", mybir.AluOpType.bypass, replica_groups=groups,
            ins=[h1_slice.opt()], outs=[h1_full.opt()],
        )

        # ======================= layer 2 =======================
        pre2 = conv_layer(2)
        sc2, sh2 = bn_coeffs(stats_sb2, st2_loc, st2_g, H2, bng2, bnb2)
        gTp = pp_s.tile([P, NG], f32, tag="st")
        for w in range(WPC):
            tp = pp_u.tile([H2, P], f32, tag="ut")
            nc.tensor.transpose(out=tp[:], in_=pre2[w][:], identity=ident[:])
            h2T = sb.tile([H2, P], f32, tag="h2T")
            nc.vector.tensor_scalar(
                out=h2T[:], in0=tp[:], scalar1=sc2[:, :1], scalar2=sh2[:, :1],
                op0=AL.mult, op1=AL.add,
            )
            bk = pp_pre.tile([P, H2], f32, tag="pre")
            nc.tensor.transpose(out=bk[:], in_=h2T[:], identity=ident[:])
            h2n = sb.tile([P, H2], f32, tag="h2n")
            nc.vector.tensor_scalar_mul(out=h2n[:], in0=bk[:], scalar1=igc[:, w : w + 1])
            ohg = sb.tile([P, NG], f32, tag="ohg")
            nc.vector.tensor_scalar(
                out=ohg[:], in0=iota256[:], scalar1=batchrel[:, w : w + 1],
                scalar2=None, op0=AL.is_equal,
            )
            nc.tensor.matmul(out=gTp[:], lhsT=h2n[:], rhs=ohg[:], start=(w == 0), stop=(w == WPC - 1))
        nc.vector.tensor_copy(out=gT_sb[:], in_=gTp[:])
        nc.sync.dma_start(out=gT_loc[:], in_=gT_sb[:])
        nc.gpsimd.collective_compute(
            "AllReduce", mybir.AluOpType.add, replica_groups=groups,
            ins=[gT_loc.opt()], outs=[gT_g.opt()],
        )
        gt = sb.tile([P, NG], f32, tag="gt")
        nc.sync.dma_start(out=gt[:], in_=gT_g[:])

        # ======================= final MLP =======================
        l1p = pp_pre.tile([H1, NG], f32, tag="pre")
        nc.tensor.matmul(out=l1p[:], lhsT=l1W[:], rhs=gt[:], start=True, stop=True)
        hl = sb.tile([H1, NG], f32, tag="hl")
        nc.vector.tensor_scalar(
            out=hl[:], in0=l1p[:], scalar1=l1b[:, :1], scalar2=0.0,
            op0=AL.add, op1=AL.max,
        )
        l2p = pp_z.tile([1, NG], f32, tag="z")
        nc.tensor.matmul(out=l2p[:], lhsT=l2W[:], rhs=hl[:], start=True, stop=True)
        osb = sb.tile([1, NG], f32, tag="osb")
        nc.vector.tensor_scalar_add(out=osb[:], in0=l2p[:], scalar1=l2b[:, :1])
        nc.sync.dma_start(out=out_d[:], in_=osb[:])

    nc.compile()
    return nc


_CACHE = {}


def _get_program(T, ES):
    key = (T, ES)
    if key not in _CACHE:
        _CACHE[key] = build_program(T, ES)
    return _CACHE[key]


def make_in_maps(inputs):
    pp = _preprocess(
        inputs["x"], inputs["edge_index"], inputs["edge_attr"], inputs["batch"]
    )
    w = _weights(inputs)
    shared = dict(
        x=np.ascontiguousarray(np.asarray(inputs["x"], np.float32)),
        W1a1=w["W1a1"], W1a2=w["W1a2"], Wp1=w["Wp1"], Wp2=w["Wp2"],
        root1=w["root1"], root2=w["root2"], bias1r=w["bias1r"], bias2r=w["bias2r"],
        bng1=w["bng1"], bnb1=w["bnb1"], bng2=w["bng2"], bnb2=w["bnb2"],
        l1W=w["l1W"], l1b=w["l1b"], l2W=w["l2W"], l2b=w["l2b"],
        iota128=w["iota128"], iota256=w["iota256"], onesP=w["onesP"], onesr=w["onesr"],
    )
    in_maps = []
    for c in range(NCORES):
        m = dict(shared)
        m["eaT"] = np.ascontiguousarray(pp["eaT"][c])
        m["srcx"] = np.ascontiguousarray(pp["srcx"][c][:, None])
        m["srch"] = np.ascontiguousarray(pp["srch"][c][:, None])
        m["dstrel"] = np.ascontiguousarray(pp["dstrel"][c][:, None])
        m["icnt"] = np.ascontiguousarray(pp["icnt"][c][:, None])
        m["xT"] = np.ascontiguousarray(pp["xT"][c])
        m["batchrel"] = np.ascontiguousarray(pp["batchrel"][c][:, None])
        m["igc"] = np.ascontiguousarray(pp["igc"][c][:, None])
        m["vmask"] = np.ascontiguousarray(pp["vmask"][c][:, None])
        in_maps.append(m)
    return in_maps, pp["T"], pp["ES"]


def _run(inputs, trace=False):
    in_maps, T, ES = make_in_maps(inputs)
    nc = _get_program(T, ES)
    res = run_bass_kernel_spmd(
        nc, in_maps, core_ids=list(range(NCORES)), trace=trace
    )
    out = np.asarray(res.results[0]["out"][0], dtype=np.float32)
    return out, res


def kernel(**inputs):
    return _run(inputs)[0]
